# revision 1
# baseline (speedup 1.0000x reference)
"""Trainium2 Bass kernel for nn_ComplexUnitaryGCN (2-layer complex unitary GCN,
circulant 16-regular graph, N=100000 nodes, D=128 dims, 8 NeuronCores).

Strategy (self-contained; shapes/sharding hardcoded):
  - Shard nodes across the 8 cores (12500 rows each) with replicated halos
    (host-side wraparound slicing) - no device-to-device communication.
  - Device works in feature-major layout: xT slab [128 dims, L+34 nodes];
    host pre-transposes the input slab and post-transposes the output.
  - The star-graph evolution row has equal weights on all 16 leaves, so the
    per-node neighbor aggregation is w0*h + wbar*(sliding window sum of 16
    neighbors along the node axis). Window sums are computed with a prefix
    scan (DVE tensor_tensor_scan) + shifted differences.
  - Layer-2 GEMM folds the layer-1 aggregation scalars into pre-scaled
    complex weight matrices (host-side), accumulating 8 real matmuls into
    PSUM per output component.
  - crelu is fused with bias via the ScalarEngine activation (Relu, bias).
"""

import numpy as np

# ---------------------------------------------------------------- constants
N = 100000
D = 128
NCORES = 8
L = N // NCORES           # 12500 nodes per core
HL, HR = 18, 18           # left/right slab halo (chain needs 18/16; +2 right
                          # so the even-padded GEMM window stays in-slab)
LH = L + HL + HR          # 12534 slab columns
CHUNK = 492               # stage-2 output chunk (CHUNK+17 <= 512 PSUM bank)
L1_CHUNK = 512            # layer-1 GEMM chunk
DEG = 16

_PROGRAM = None           # cached (nc, names) - compile once per process


# ------------------------------------------------------------- host helpers
def _evolution_row(deg, tr, ti):
    """Replicate reference._evolution_row (jax f32 on CPU when available)."""
    try:
        import jax

        cpu = jax.devices("cpu")[0]
        with jax.default_device(cpu):
            import jax.numpy as jnp

            n = deg + 1
            A = jnp.zeros((n, n), jnp.complex64).at[0, 1:].set(1.0).at[1:, 0].set(1.0)
            t = (jnp.float32(tr) + 1j * jnp.float32(ti)).astype(jnp.complex64)
            G = jax.scipy.linalg.expm(-1j * A * t)
            s = jnp.sqrt(jnp.max(jnp.linalg.eigvalsh(G @ G.conj().T))).astype(
                jnp.complex64
            )
            Lt = G / s
            Rt = jnp.sqrt(jnp.eye(n, dtype=jnp.complex64) - Lt @ (G.conj().T / s))
            return np.asarray(Lt[0] + Rt[0])
    except Exception:
        n = deg + 1
        A = np.zeros((n, n), np.float64)
        A[0, 1:] = 1.0
        A[1:, 0] = 1.0
        t = complex(tr, ti)
        evals, evecs = np.linalg.eigh(A)
        G = (evecs * np.exp(-1j * evals * t)) @ evecs.T
        s = np.sqrt(np.max(np.linalg.eigvalsh(G @ G.conj().T)))
        Lt = G / s
        Rt = np.sqrt(np.eye(n) - Lt @ (G.conj().T / s))
        return (Lt[0] + Rt[0]).astype(np.complex64)


def _fold_weights(ins, w1, w2):
    """Pre-scale/transpose all weights into the device layouts (f32)."""
    W1r, W1i = ins["W1r"], ins["W1i"]
    W2c = ins["W2r"] + 1j * ins["W2i"]
    b2c = ins["b2r"] + 1j * ins["b2i"]
    eb1c = ins["eb1r"] + 1j * ins["eb1i"]
    w0_1, wb_1 = w1[0], w1[1:].mean()
    w0_2, wb_2 = w2[0], w2[1:].mean()
    Wa = (w0_1 - wb_1) * W2c          # layer-2 direct-h1 term
    Wb = wb_1 * W2c                   # layer-2 window-sum term
    b2p = b2c + W2c @ eb1c            # eb1 folded through GEMM2
    c_h2r = (w0_2 - wb_2).real
    c_h2i = -(w0_2 - wb_2).imag
    c_nr = wb_2.real
    c_ni = -wb_2.imag
    c0 = c_h2r
    f32 = np.float32
    # lhsT layout: [K=feat_in partitions, M=feat_out] == numpy transpose of [out,in]
    wl1 = np.concatenate([W1r.T, W1i.T], axis=1).astype(f32)          # [128, 256]
    wg = np.concatenate(
        [
            Wa.real.T, -Wa.imag.T, Wb.real.T, -Wb.imag.T,             # -> gr
            Wa.imag.T, Wa.real.T, Wb.imag.T, Wb.real.T,               # -> gi
        ],
        axis=1,
    ).astype(f32)                                                      # [128, 1024]
    biases = np.stack(
        [
            ins["b1r"], ins["b1i"], b2p.real, b2p.imag, ins["eb2r"],
            np.zeros(D), np.zeros(D), np.zeros(D),
        ],
        axis=1,
    ).astype(f32)                                                      # [128, 8]
    # layer-2 window term: c_nr*(P2r[+8]-P2r[-9]) + c_ni*(P2i[+8]-P2i[-9]).
    # scan is linear, so scan(alpha*h2r + beta*h2i) = alpha*P2r + beta*P2i:
    # one combined scan with (alpha, beta) prop-to (c_nr, c_ni), normalized by
    # the larger coefficient for conditioning.
    if max(abs(c_nr), abs(c_ni)) == 0.0:
        h2c_on_r, h2c_scale, q = False, 0.0, 0.0
    elif abs(c_nr) >= abs(c_ni):
        h2c_on_r, h2c_scale, q = False, c_ni / c_nr, c_nr   # u = P2r + s*P2i
    else:
        h2c_on_r, h2c_scale, q = True, c_nr / c_ni, c_ni    # u = s*P2r + P2i
    scalars = dict(
        r1=float(c_h2i / c0), c0=float(c0),
        h2c_on_r=bool(h2c_on_r), h2c_scale=float(h2c_scale), qn=float(q / c0),
    )
    return wl1, wg, biases, scalars


def _is_circulant(edge_index):
    """Check edge_index matches the reference's circulant construction."""
    if edge_index.shape != (2, N * DEG // 2):
        return False
    K = DEG // 2
    i = np.arange(N)
    src = np.repeat(i, K)
    dst = ((i[:, None] + np.arange(1, K + 1)[None, :]) % N).reshape(-1)
    return bool(
        np.array_equal(edge_index[0], src) and np.array_equal(edge_index[1], dst)
    )


def _fallback_numpy(ins):
    """Exact reference semantics on host (any edge_index). Slow but correct."""
    x = ins["x"]
    edge_index = ins["edge_index"]
    src, dst = edge_index[0], edge_index[1]
    nodes = np.concatenate([src, dst])
    nbr = np.concatenate([dst, src])
    order = np.lexsort((nbr, nodes))
    deg = nodes.shape[0] // N
    nbrs = nbr[order].reshape(N, deg)
    h = x.astype(np.complex64)

    def crelu(z):
        return (np.maximum(z.real, 0) + 1j * np.maximum(z.imag, 0)).astype(
            np.complex64
        )

    for l in ("1", "2"):
        W = (ins[f"W{l}r"] + 1j * ins[f"W{l}i"]).astype(np.complex64)
        b = (ins[f"b{l}r"] + 1j * ins[f"b{l}i"]).astype(np.complex64)
        h = crelu(h @ W.T + b)
        w = _evolution_row(deg, float(ins[f"t{l}r"]), float(ins[f"t{l}i"]))
        out = w[0] * h
        for k in range(deg):
            out = out + w[1 + k] * h[nbrs[:, k]]
        h = (out + (ins[f"eb{l}r"] + 1j * ins[f"eb{l}i"])).astype(np.complex64)
    return np.ascontiguousarray(h.real.astype(np.float32))


# ------------------------------------------------------------ device program
def _build_program(matmul_dtype="float32r", reps=1):
    import concourse.bacc as bacc
    import concourse.bass as bass
    import concourse.mybir as mybir
    import concourse.tile as tile

    f32 = mybir.dt.float32
    mm_dt = getattr(mybir.dt, matmul_dtype)
    AF = mybir.ActivationFunctionType
    OP = mybir.AluOpType

    nc = bacc.Bacc("TRN2", target_bir_lowering=False, debug=False)

    xT = nc.dram_tensor("xT", [D, LH], mm_dt, kind="ExternalInput")
    wl1_d = nc.dram_tensor("wl1", [D, 2 * D], mm_dt, kind="ExternalInput")
    wg_d = nc.dram_tensor("wg", [D, 8 * D], mm_dt, kind="ExternalInput")
    bias_d = nc.dram_tensor("biases", [D, 8], f32, kind="ExternalInput")
    sc_d = nc.dram_tensor("scalars", [1, 8], f32, kind="ExternalInput")
    outT = nc.dram_tensor("outT", [D, L], f32, kind="ExternalOutput")

    # python-side scalars are baked as immediates; sc_d exists only so the
    # in_maps are uniform if we ever need runtime scalars (unused for now).
    del sc_d

    with tile.TileContext(nc) as tc:
        with (
            tc.tile_pool(name="consts", bufs=1) as cpool,
            tc.tile_pool(name="slab", bufs=1) as slab,
            tc.tile_pool(name="xs", bufs=3) as xs,
            tc.tile_pool(name="ps1", bufs=2, space="PSUM") as ps1,
            tc.tile_pool(name="ps2", bufs=2, space="PSUM") as ps2,
            tc.tile_pool(name="st2", bufs=3) as st2,
            tc.tile_pool(name="outp", bufs=3) as outp,
        ):
            wl1 = cpool.tile([D, 2 * D], mm_dt)
            wg = cpool.tile([D, 8 * D], mm_dt)
            bias = cpool.tile([D, 8], f32)
            nc.sync.dma_start(wl1[:], wl1_d[:])
            nc.sync.dma_start(wg[:], wg_d[:])
            nc.sync.dma_start(bias[:], bias_d[:])

            for _rep in range(reps):
              h1r = slab.tile([D, LH], mm_dt, tag="h1r")
              h1i = slab.tile([D, LH], mm_dt, tag="h1i")

              # ---- layer 1: h1 = crelu(W1 @ xT + b1) over the whole slab
              n1 = (LH + L1_CHUNK - 1) // L1_CHUNK
              for k in range(n1):
                  s = k * L1_CHUNK
                  cw = min(L1_CHUNK, LH - s)
                  xt = xs.tile([D, L1_CHUNK], mm_dt)
                  nc.sync.dma_start(xt[:, :cw], xT[:, s : s + cw])
                  pr = ps1.tile([D, L1_CHUNK], f32, tag="ps1r")
                  pi = ps1.tile([D, L1_CHUNK], f32, tag="ps1i")
                  nc.tensor.matmul(
                      pr[:, :cw],
                      wl1[:, 0:D],
                      xt[:, :cw],
                      start=True,
                      stop=True,
                  )
                  nc.tensor.matmul(
                      pi[:, :cw],
                      wl1[:, D : 2 * D],
                      xt[:, :cw],
                      start=True,
                      stop=True,
                  )
                  nc.scalar.activation(
                      h1r[:, s : s + cw], pr[:, :cw], AF.Relu, bias=bias[:, 0:1]
                  )
                  nc.scalar.activation(
                      h1i[:, s : s + cw], pi[:, :cw], AF.Relu, bias=bias[:, 1:2]
                  )

              # ---- stage 2: per out-chunk [a, a+cw) in slab coords
              n2 = (L + CHUNK - 1) // CHUNK
              for k in range(n2):
                  a = HL + k * CHUNK
                  cw = min(CHUNK, L - k * CHUNK)
                  # fp32r matmuls require even moving/dst free sizes -> pad
                  # the GEMM window to even (extra col computed, never read)
                  w2n = cw + 17 + ((cw + 17) % 2)
                  w1s, w1n = a - 18, w2n + 17
                  w2s = a - 9

                  # DVE reads of the mm-dtype h1 slab: bitcast f32r -> f32
                  # (identical bits); bf16 reads convert natively.
                  dve_view = (
                      (lambda ap: ap.bitcast(f32))
                      if matmul_dtype == "float32r"
                      else (lambda ap: ap)
                  )
                  p1r = st2.tile([D, CHUNK + 36], f32, tag="p1r")
                  p1i = st2.tile([D, CHUNK + 36], f32, tag="p1i")
                  nc.vector.tensor_tensor_scan(
                      p1r[:, :w1n],
                      dve_view(h1r[:, w1s : w1s + w1n]),
                      dve_view(h1r[:, w1s : w1s + w1n]),
                      0.0,
                      OP.add,
                      OP.bypass,
                  )
                  nc.vector.tensor_tensor_scan(
                      p1i[:, :w1n],
                      dve_view(h1i[:, w1s : w1s + w1n]),
                      dve_view(h1i[:, w1s : w1s + w1n]),
                      0.0,
                      OP.add,
                      OP.bypass,
                  )
                  ns1r = st2.tile([D, CHUNK + 18], mm_dt, tag="ns1r")
                  ns1i = st2.tile([D, CHUNK + 18], mm_dt, tag="ns1i")
                  # NS1'[j] = P1[j+8] - P1[j-9]   (window-17 sum, center incl.)
                  nc.vector.scalar_tensor_tensor(
                      ns1r[:, :w2n],
                      p1r[:, 17 : 17 + w2n],
                      1.0,
                      p1r[:, 0:w2n],
                      OP.mult,
                      OP.subtract,
                  )
                  nc.vector.scalar_tensor_tensor(
                      ns1i[:, :w2n],
                      p1i[:, 17 : 17 + w2n],
                      1.0,
                      p1i[:, 0:w2n],
                      OP.mult,
                      OP.subtract,
                  )

                  pgr = ps2.tile([D, CHUNK + 18], f32, tag="ps2r")
                  pgi = ps2.tile([D, CHUNK + 18], f32, tag="ps2i")
                  rhs_list = [
                      h1r[:, w2s : w2s + w2n],
                      h1i[:, w2s : w2s + w2n],
                      ns1r[:, :w2n],
                      ns1i[:, :w2n],
                  ]
                  for comp, ptile in ((0, pgr), (1, pgi)):
                      for t_i, rhs in enumerate(rhs_list):
                          wcol = (comp * 4 + t_i) * D
                          nc.tensor.matmul(
                              ptile[:, :w2n],
                              wg[:, wcol : wcol + D],
                              rhs,
                              start=(t_i == 0),
                              stop=(t_i == 3),
                          )

                  h2r = st2.tile([D, CHUNK + 18], f32, tag="h2r")
                  h2i = st2.tile([D, CHUNK + 18], f32, tag="h2i")
                  nc.scalar.activation(
                      h2r[:, :w2n], pgr[:, :w2n], AF.Relu, bias=bias[:, 2:3]
                  )
                  nc.scalar.activation(
                      h2i[:, :w2n], pgi[:, :w2n], AF.Relu, bias=bias[:, 3:4]
                  )

                  # out = c0*(h2r + r1*h2i) + q*(u[+8]-u[-9]) + eb2r, where
                  # u = scan(alpha*h2r + beta*h2i) -- one scan instead of two
                  sc = _build_program.scalars
                  h2c = st2.tile([D, CHUNK + 18], f32, tag="h2c")
                  if sc["h2c_on_r"]:
                      nc.vector.scalar_tensor_tensor(
                          h2c[:, :w2n], h2r[:, :w2n], sc["h2c_scale"],
                          h2i[:, :w2n], OP.mult, OP.add,
                      )
                  else:
                      nc.vector.scalar_tensor_tensor(
                          h2c[:, :w2n], h2i[:, :w2n], sc["h2c_scale"],
                          h2r[:, :w2n], OP.mult, OP.add,
                      )
                  u = st2.tile([D, CHUNK + 18], f32, tag="u")
                  nc.vector.tensor_tensor_scan(
                      u[:, :w2n], h2c[:, :w2n], h2c[:, :w2n], 0.0, OP.add, OP.bypass
                  )
                  t1 = st2.tile([D, CHUNK], f32, tag="t1")
                  t2 = st2.tile([D, CHUNK], f32, tag="t2")
                  t3 = st2.tile([D, CHUNK], f32, tag="t3")
                  ot = outp.tile([D, CHUNK], f32)
                  nc.vector.scalar_tensor_tensor(
                      t1[:, :cw], h2i[:, 9 : 9 + cw], sc["r1"], h2r[:, 9 : 9 + cw],
                      OP.mult, OP.add,
                  )
                  nc.vector.scalar_tensor_tensor(
                      t2[:, :cw], u[:, 17 : 17 + cw], sc["qn"], t1[:, :cw],
                      OP.mult, OP.add,
                  )
                  nc.vector.scalar_tensor_tensor(
                      t3[:, :cw], u[:, 0:cw], -sc["qn"], t2[:, :cw],
                      OP.mult, OP.add,
                  )
                  nc.vector.tensor_scalar(
                      ot[:, :cw], t3[:, :cw], sc["c0"], bias[:, 4:5],
                      OP.mult, OP.add,
                  )
                  nc.sync.dma_start(outT[:, k * CHUNK : k * CHUNK + cw], ot[:, :cw])

    nc.compile()
    return nc


_MM_DTYPE = "float32r"  # fast PE path; auto-falls back to float32


def _get_program(scalars):
    global _PROGRAM
    _build_program.scalars = scalars
    if _PROGRAM is None:
        _PROGRAM = _build_program(_MM_DTYPE)
    return _PROGRAM


def _reset_program(mm_dtype):
    global _MM_DTYPE, _PROGRAM, _EXEC
    _MM_DTYPE = mm_dtype
    _PROGRAM = None
    _EXEC = None


# ------------------------------------------------------- cached PJRT runner
_EXEC = None  # (sharded_fn, in_names, out_names, out_avals, n_params)


def _get_executable(nc):
    """Build (once) a jitted shard_map executable for the 8-core SPMD run,
    mirroring concourse.bass2jax.run_bass_via_pjrt but cached so repeat
    calls don't re-trace/re-compile."""
    global _EXEC
    if _EXEC is not None:
        return _EXEC
    import jax
    import numpy as _np
    from jax.sharding import Mesh, PartitionSpec
    from jax.experimental.shard_map import shard_map

    import concourse.mybir as mybir
    from concourse import bass2jax

    bass2jax.install_neuronx_cc_hook()

    partition_name = (
        nc.partition_id_tensor.name if nc.partition_id_tensor else None
    )
    in_names, out_names, out_avals = [], [], []
    for alloc in nc.m.functions[0].allocations:
        if not isinstance(alloc, mybir.MemoryLocationSet):
            continue
        name = alloc.memorylocations[0].name
        if alloc.kind == "ExternalInput":
            if name != partition_name:
                in_names.append(name)
        elif alloc.kind == "ExternalOutput":
            out_names.append(name)
            out_avals.append(
                jax.core.ShapedArray(
                    tuple(alloc.tensor_shape), mybir.dt.np(alloc.dtype)
                )
            )
    n_params = len(in_names)
    all_names = in_names + out_names
    if partition_name is not None:
        all_names = all_names + [partition_name]

    def _body(*args):
        operands = list(args)
        if partition_name is not None:
            operands.append(bass2jax.partition_id_tensor())
        outs = bass2jax._bass_exec_p.bind(
            *operands,
            out_avals=tuple(out_avals),
            in_names=tuple(all_names),
            out_names=tuple(out_names),
            lowering_input_output_aliases=(),
            sim_require_finite=True,
            sim_require_nnan=True,
            nc=nc,
        )
        return tuple(outs)

    devices = jax.devices()[:NCORES]
    mesh = Mesh(_np.asarray(devices), ("core",))
    in_specs = (PartitionSpec("core"),) * (n_params + len(out_names))
    out_specs = (PartitionSpec("core"),) * len(out_names)
    donate = tuple(range(n_params, n_params + len(out_names)))
    sharded = jax.jit(
        shard_map(
            _body, mesh=mesh, in_specs=in_specs, out_specs=out_specs, check_rep=False
        ),
        donate_argnums=donate,
        keep_unused=True,
    )
    _EXEC = (sharded, in_names, out_names, out_avals, n_params)
    return _EXEC


def _execute(in_maps):
    import jax.numpy as jnp

    nc = _PROGRAM
    sharded, in_names, out_names, out_avals, n_params = _get_executable(nc)
    concat_in = [
        np.concatenate([m[name] for m in in_maps], axis=0) for name in in_names
    ]
    zeros = [
        jnp.zeros((NCORES * a.shape[0], *a.shape[1:]), a.dtype) for a in out_avals
    ]
    out_arrs = sharded(*concat_in, *zeros)
    return {
        name: np.asarray(out_arrs[i]).reshape(NCORES, *out_avals[i].shape)
        for i, name in enumerate(out_names)
    }


# ---------------------------------------------------------------- entrypoint
def _prepare(ins):
    ins = {k: np.asarray(v) for k, v in ins.items()}
    w1 = _evolution_row(DEG, float(ins["t1r"]), float(ins["t1i"]))
    w2 = _evolution_row(DEG, float(ins["t2r"]), float(ins["t2i"]))
    wl1, wg, biases, scalars = _fold_weights(ins, w1, w2)
    _get_program(scalars)

    x = ins["x"].astype(np.float32, copy=False)
    idx = np.arange(-HL, L + HR)
    sc_arr = np.zeros((1, 8), np.float32)
    in_maps = []
    for c in range(NCORES):
        rows = (c * L + idx) % N
        xTslab = np.ascontiguousarray(x[rows].T)
        in_maps.append(
            {"xT": xTslab, "wl1": wl1, "wg": wg, "biases": biases, "scalars": sc_arr}
        )
    return in_maps


def _run(ins, trace=False):
    ins = {k: np.asarray(v) for k, v in ins.items()}
    if not _is_circulant(ins["edge_index"]):
        return _fallback_numpy(ins), None
    in_maps = _prepare(ins)
    try:
        outs = _execute(in_maps)
    except Exception:
        if _MM_DTYPE == "float32":
            raise
        # fp32r path rejected by the compiler on this stack - retry in fp32
        _reset_program("float32")
        in_maps = _prepare(ins)
        outs = _execute(in_maps)
    out = np.empty((N, D), np.float32)
    for c in range(NCORES):
        out[c * L : (c + 1) * L] = outs["outT"][c].T
    return out, None


def kernel(**inputs):
    out, _ = _run(inputs)
    return out



# revision 4
# speedup vs baseline: 745.9337x; 745.9337x over previous
"""Trainium2 Bass kernel for nn_ComplexUnitaryGCN (2-layer complex unitary GCN,
circulant 16-regular graph, N=100000 nodes, D=128 dims, 8 NeuronCores).

v2 pipeline (primary):
  - Nodes sharded across 8 cores with replicated halos; per-core input is a
    host-built [12544, 128] bf16 slab (3 contiguous memcpys per core).
  - On device, the slab is transposed feature-major by the DMA XBAR
    (16x128-tile transpose DMA, bf16) straight into SBUF.
  - Layer 1 (h1 = crelu(W1 x + b1)) is pointwise in nodes: bf16 GEMM chunks.
  - Key restructure vs v1: the layer-1 star aggregation commutes with the
    layer-2 GEMM, so compute C = W2c @ h1 FIRST (4 bf16 matmuls / chunk
    instead of 8) and apply  alpha*C + beta*window17(C)  elementwise.
    Window sums come from one prefix scan per component (DVE/Pool), with
    combos normalized by the dominant scalar component for conditioning.
  - The layer-2 aggregation + real projection is the v1 scan trick.
  - Elementwise work is split across DVE / GpSimd / ScalarE; output is
    transposed back node-major by XBAR SBUF->SBUF block transposes and
    leaves as bf16 [12500, 128] per core.
  - Host: cached circulant check, cached evolution rows, cached device
    weights + output containers; only x (25MB bf16) moves per call.
Falls back to the v1 f32r program, then f32, then a numpy reference.
"""

import numpy as np

# ---------------------------------------------------------------- constants
N = 100000
D = 128
NCORES = 8
L = N // NCORES           # 12500 nodes per core
DEG = 16

# ---- v2 geometry
HL2, HR2 = 24, 20         # halos; LH2 must be a multiple of 16 for the XBAR
LH2 = L + HL2 + HR2       # 12544 slab rows
L1C = 1024                # layer-1 chunk (two 512 PSUM halves per component)
CH2 = 464                 # stage-2 output chunk; n1 = CH2+36 = 500 <= 512
NCH2 = 27                 # 26*464 + 436 = 12500

# ---- v1 geometry (legacy fallback)
HL, HR = 18, 18
LH = L + HL + HR          # 12534
CHUNK = 492
L1_CHUNK = 512

_PROGRAM = None           # legacy compiled program
_P2 = None                # v2 compiled program
_P2_scalars = None        # scalars baked into _P2


# ------------------------------------------------------------- host helpers
def _evolution_row_impl(deg, tr, ti):
    try:
        import jax

        cpu = jax.devices("cpu")[0]
        with jax.default_device(cpu):
            import jax.numpy as jnp

            n = deg + 1
            A = jnp.zeros((n, n), jnp.complex64).at[0, 1:].set(1.0).at[1:, 0].set(1.0)
            t = (jnp.float32(tr) + 1j * jnp.float32(ti)).astype(jnp.complex64)
            G = jax.scipy.linalg.expm(-1j * A * t)
            s = jnp.sqrt(jnp.max(jnp.linalg.eigvalsh(G @ G.conj().T))).astype(
                jnp.complex64
            )
            Lt = G / s
            Rt = jnp.sqrt(jnp.eye(n, dtype=jnp.complex64) - Lt @ (G.conj().T / s))
            return np.asarray(Lt[0] + Rt[0])
    except Exception:
        n = deg + 1
        A = np.zeros((n, n), np.float64)
        A[0, 1:] = 1.0
        A[1:, 0] = 1.0
        t = complex(tr, ti)
        evals, evecs = np.linalg.eigh(A)
        G = (evecs * np.exp(-1j * evals * t)) @ evecs.T
        s = np.sqrt(np.max(np.linalg.eigvalsh(G @ G.conj().T)))
        Lt = G / s
        Rt = np.sqrt(np.eye(n) - Lt @ (G.conj().T / s))
        return (Lt[0] + Rt[0]).astype(np.complex64)


_EVO_CACHE = {}


def _evolution_row(deg, tr, ti):
    key = (deg, float(tr), float(ti))
    if key not in _EVO_CACHE:
        _EVO_CACHE[key] = _evolution_row_impl(deg, tr, ti)
    return _EVO_CACHE[key]


_CIRC_EXPECT = None


def _is_circulant(edge_index):
    """Check edge_index matches the reference's circulant construction."""
    global _CIRC_EXPECT
    if edge_index.shape != (2, N * DEG // 2):
        return False
    if _CIRC_EXPECT is None:
        K = DEG // 2
        i = np.arange(N, dtype=edge_index.dtype)
        src = np.repeat(i, K)
        dst = ((i[:, None] + np.arange(1, K + 1, dtype=edge_index.dtype)[None, :])
               % N).reshape(-1)
        _CIRC_EXPECT = np.stack([src, dst]).astype(np.int32)
    return bool(np.array_equal(edge_index, _CIRC_EXPECT))


def _fallback_numpy(ins):
    """Exact reference semantics on host (any edge_index). Slow but correct."""
    x = ins["x"]
    edge_index = ins["edge_index"]
    src, dst = edge_index[0], edge_index[1]
    nodes = np.concatenate([src, dst])
    nbr = np.concatenate([dst, src])
    order = np.lexsort((nbr, nodes))
    deg = nodes.shape[0] // N
    nbrs = nbr[order].reshape(N, deg)
    h = x.astype(np.complex64)

    def crelu(z):
        return (np.maximum(z.real, 0) + 1j * np.maximum(z.imag, 0)).astype(
            np.complex64
        )

    for l in ("1", "2"):
        W = (ins[f"W{l}r"] + 1j * ins[f"W{l}i"]).astype(np.complex64)
        b = (ins[f"b{l}r"] + 1j * ins[f"b{l}i"]).astype(np.complex64)
        h = crelu(h @ W.T + b)
        w = _evolution_row(deg, float(ins[f"t{l}r"]), float(ins[f"t{l}i"]))
        out = w[0] * h
        for k in range(deg):
            out = out + w[1 + k] * h[nbrs[:, k]]
        h = (out + (ins[f"eb{l}r"] + 1j * ins[f"eb{l}i"]).astype(np.complex64))
        h = h.astype(np.complex64)
    return np.ascontiguousarray(h.real.astype(np.float32))


class _Degenerate(Exception):
    pass


# ----------------------------------------------------------- v2 weight fold
def _fold2(ins, w1, w2):
    """Device layouts + baked scalars for the v2 program."""
    import ml_dtypes

    bf16 = np.dtype(ml_dtypes.bfloat16)
    W1r, W1i = ins["W1r"], ins["W1i"]
    W2r, W2i = ins["W2r"], ins["W2i"]
    W2c = W2r + 1j * W2i
    b2c = ins["b2r"] + 1j * ins["b2i"]
    eb1c = ins["eb1r"] + 1j * ins["eb1i"]
    b2p = b2c + W2c @ eb1c

    alpha = complex(w1[0] - w1[1:].mean())
    beta = complex(w1[1:].mean())
    a2 = complex(w2[0] - w2[1:].mean())
    b2s = complex(w2[1:].mean())

    # wz cols: W1rT | W1iT | W2rT | W2iT | -W2iT   (lhsT layout [in, out])
    wz = np.concatenate(
        [W1r.T, W1i.T, W2r.T, W2i.T, -W2i.T], axis=1
    ).astype(bf16)                                               # [128, 640]
    biases = np.stack(
        [
            ins["b1r"], ins["b1i"], b2p.real.astype(np.float32),
            b2p.imag.astype(np.float32), ins["eb2r"],
            np.zeros(D, np.float32), np.zeros(D, np.float32),
            np.zeros(D, np.float32),
        ],
        axis=1,
    ).astype(np.float32)                                         # [128, 8]

    mag = abs(alpha) + abs(beta)
    # window-17 combos normalized by the dominant component of beta / alpha
    if abs(beta.real) >= abs(beta.imag):
        B_s = beta.real
        s_ur, ur_form = (-beta.imag / B_s if B_s else 0.0), "cis_first_add"
        s_ui, ui_form = (beta.imag / B_s if B_s else 0.0), "cr_first_add"
    else:
        B_s = beta.imag
        s_ur, ur_form = beta.real / B_s, "cr_first_sub"
        s_ui, ui_form = beta.real / B_s, "cis_first_add"
    if abs(alpha.real) >= abs(alpha.imag):
        A_s = alpha.real
        if abs(A_s) < 1e-12 * (mag + 1e-30):
            raise _Degenerate("alpha ~ 0")
        s_vr, vr_form = -alpha.imag / A_s, "cis_first_add"
        s_vi, vi_form = alpha.imag / A_s, "cr_first_add"
    else:
        A_s = alpha.imag
        s_vr, vr_form = alpha.real / A_s, "cr_first_sub"
        s_vi, vi_form = alpha.real / A_s, "cis_first_add"
    gam = (B_s / A_s) if B_s else 0.0

    c_h2r, c_h2i = a2.real, -a2.imag
    c_nr, c_ni = b2s.real, -b2s.imag
    c0 = c_h2r
    if abs(c0) < 1e-12 * (abs(a2) + abs(b2s) + 1e-30):
        raise _Degenerate("c0 ~ 0")
    r1 = c_h2i / c0
    if max(abs(c_nr), abs(c_ni)) == 0.0:
        h2c_on_r, h2c_scale, q = False, 0.0, 0.0
    elif abs(c_nr) >= abs(c_ni):
        h2c_on_r, h2c_scale, q = False, c_ni / c_nr, c_nr
    else:
        h2c_on_r, h2c_scale, q = True, c_nr / c_ni, c_ni
    scalars = dict(
        s_ur=float(s_ur), ur_form=ur_form, s_ui=float(s_ui), ui_form=ui_form,
        s_vr=float(s_vr), vr_form=vr_form, s_vi=float(s_vi), vi_form=vi_form,
        gam=float(gam), A_s=float(A_s),
        r1=float(r1), c0=float(c0),
        h2c_on_r=bool(h2c_on_r), h2c_scale=float(h2c_scale),
        qn=float(q / c0),
    )
    return wz, biases, scalars


# ------------------------------------------------------------ v2 device program
def _build_program2(scalars, reps=1):
    import concourse.bacc as bacc
    import concourse.mybir as mybir
    import concourse.tile as tile

    f32 = mybir.dt.float32
    bf16 = mybir.dt.bfloat16
    AF = mybir.ActivationFunctionType
    OP = mybir.AluOpType
    sc = scalars

    nc = bacc.Bacc("TRN2", target_bir_lowering=False, debug=False)

    xs_d = nc.dram_tensor("xs", [LH2, D], bf16, kind="ExternalInput")
    wz_d = nc.dram_tensor("wz", [D, 5 * D], bf16, kind="ExternalInput")
    bz_d = nc.dram_tensor("bz", [D, 8], f32, kind="ExternalInput")
    out_d = nc.dram_tensor("outN", [L, D], bf16, kind="ExternalOutput")

    def combo(eng, out_ap, form, s, cr_ap, cis_ap):
        """out = alpha-normalized linear combo of (Cr, Ci)."""
        if form == "cis_first_add":      # (cis * s) + Cr
            eng.scalar_tensor_tensor(out_ap, cis_ap, s, cr_ap, OP.mult, OP.add)
        elif form == "cr_first_add":     # (Cr * s) + cis
            eng.scalar_tensor_tensor(out_ap, cr_ap, s, cis_ap, OP.mult, OP.add)
        elif form == "cr_first_sub":     # (Cr * s) - cis
            eng.scalar_tensor_tensor(out_ap, cr_ap, s, cis_ap, OP.mult, OP.subtract)
        else:
            raise ValueError(form)

    with tile.TileContext(nc) as tc:
        with (
            tc.tile_pool(name="consts", bufs=1) as cpool,
            tc.tile_pool(name="slab", bufs=1) as slab,
            tc.tile_pool(name="xp", bufs=2) as xp,
            tc.tile_pool(name="l1p", bufs=1, space="PSUM") as l1p,
            tc.tile_pool(name="cps", bufs=2, space="PSUM") as cps,
            tc.tile_pool(name="sp", bufs=2) as sp,
            tc.tile_pool(name="op", bufs=3) as op_,
        ):
            wz = cpool.tile([D, 5 * D], bf16)
            bz = cpool.tile([D, 8], f32)
            nc.sync.dma_start(wz[:], wz_d[:])
            nc.sync.dma_start(bz[:], bz_d[:])

            for _rep in range(reps):
                h1r = slab.tile([D, LH2], bf16, tag="h1r")
                h1i = slab.tile([D, LH2], bf16, tag="h1i")

                # ---- layer 1 over the whole slab (XBAR-transposed loads)
                nl1 = (LH2 + L1C - 1) // L1C
                for j in range(nl1):
                    s0 = j * L1C
                    cols = min(L1C, LH2 - s0)
                    xt = xp.tile([D, L1C], bf16)
                    nc.sync.dma_start(
                        xt[:, :cols], xs_d[s0 : s0 + cols, :], transpose=True
                    )
                    for hh in range(0, cols, 512):
                        w = min(512, cols - hh)
                        tg = hh // 512
                        pr = l1p.tile([D, 512], f32, tag=f"pr{tg}")
                        pi = l1p.tile([D, 512], f32, tag=f"pi{tg}")
                        nc.tensor.matmul(
                            pr[:, :w], wz[:, 0:D], xt[:, hh : hh + w],
                            start=True, stop=True,
                        )
                        nc.tensor.matmul(
                            pi[:, :w], wz[:, D : 2 * D], xt[:, hh : hh + w],
                            start=True, stop=True,
                        )
                        nc.scalar.activation(
                            h1r[:, s0 + hh : s0 + hh + w], pr[:, :w],
                            AF.Relu, bias=bz[:, 0:1],
                        )
                        nc.scalar.activation(
                            h1i[:, s0 + hh : s0 + hh + w], pi[:, :w],
                            AF.Relu, bias=bz[:, 1:2],
                        )

                # ---- stage 2: per out-chunk [a, a+cw) in slab coords
                for k in range(NCH2):
                    a = HL2 + k * CH2
                    cw = min(CH2, L - k * CH2)
                    o1 = a - 18
                    n1 = cw + 36
                    n2 = cw + 18
                    # C = W2c @ h1 over [o1, o1+n1)
                    cr = cps.tile([D, 512], f32, tag="cr")
                    ci = cps.tile([D, 512], f32, tag="ci")
                    r_sl = h1r[:, o1 : o1 + n1]
                    i_sl = h1i[:, o1 : o1 + n1]
                    nc.tensor.matmul(
                        cr[:, :n1], wz[:, 2 * D : 3 * D], r_sl,
                        start=True, stop=False,
                    )
                    nc.tensor.matmul(
                        cr[:, :n1], wz[:, 4 * D : 5 * D], i_sl,
                        start=False, stop=True,
                    )
                    nc.tensor.matmul(
                        ci[:, :n1], wz[:, 3 * D : 4 * D], r_sl,
                        start=True, stop=False,
                    )
                    nc.tensor.matmul(
                        ci[:, :n1], wz[:, 2 * D : 3 * D], i_sl,
                        start=False, stop=True,
                    )
                    # GpSimd cannot read PSUM: stage both C components to SBUF
                    crs = sp.tile([D, 512], bf16, tag="crs")
                    cis = sp.tile([D, 512], bf16, tag="cis")
                    nc.scalar.copy(crs[:, :n1], cr[:, :n1])
                    nc.scalar.copy(cis[:, :n1], ci[:, :n1])

                    u1r = sp.tile([D, 512], bf16, tag="u1r")
                    u1i = sp.tile([D, 512], bf16, tag="u1i")
                    combo(nc.vector, u1r[:, :n1], sc["ur_form"], sc["s_ur"],
                          crs[:, :n1], cis[:, :n1])
                    combo(nc.vector, u1i[:, :n1], sc["ui_form"], sc["s_ui"],
                          crs[:, :n1], cis[:, :n1])
                    p1r = sp.tile([D, 512], f32, tag="p1r")
                    p1i = sp.tile([D, 512], f32, tag="p1i")
                    nc.vector.tensor_tensor_scan(
                        p1r[:, :n1], u1r[:, :n1], u1r[:, :n1], 0.0,
                        OP.add, OP.bypass,
                    )
                    nc.vector.tensor_tensor_scan(
                        p1i[:, :n1], u1i[:, :n1], u1i[:, :n1], 0.0,
                        OP.add, OP.bypass,
                    )
                    d_r = sp.tile([D, 512], bf16, tag="d_r")
                    d_i = sp.tile([D, 512], bf16, tag="d_i")
                    nc.vector.scalar_tensor_tensor(
                        d_r[:, :n2], p1r[:, 17 : 17 + n2], 1.0, p1r[:, 0:n2],
                        OP.mult, OP.subtract,
                    )
                    nc.vector.scalar_tensor_tensor(
                        d_i[:, :n2], p1i[:, 17 : 17 + n2], 1.0, p1i[:, 0:n2],
                        OP.mult, OP.subtract,
                    )
                    v1r = sp.tile([D, 512], bf16, tag="v1r")
                    v1i = sp.tile([D, 512], bf16, tag="v1i")
                    combo(nc.vector, v1r[:, :n2], sc["vr_form"], sc["s_vr"],
                          crs[:, 9 : 9 + n2], cis[:, 9 : 9 + n2])
                    combo(nc.vector, v1i[:, :n2], sc["vi_form"], sc["s_vi"],
                          crs[:, 9 : 9 + n2], cis[:, 9 : 9 + n2])
                    z1r = sp.tile([D, 512], bf16, tag="z1r")
                    z1i = sp.tile([D, 512], bf16, tag="z1i")
                    nc.vector.scalar_tensor_tensor(
                        z1r[:, :n2], d_r[:, :n2], sc["gam"], v1r[:, :n2],
                        OP.mult, OP.add,
                    )
                    nc.vector.scalar_tensor_tensor(
                        z1i[:, :n2], d_i[:, :n2], sc["gam"], v1i[:, :n2],
                        OP.mult, OP.add,
                    )
                    h2r = sp.tile([D, 512], bf16, tag="h2r")
                    h2i = sp.tile([D, 512], bf16, tag="h2i")
                    nc.scalar.activation(
                        h2r[:, :n2], z1r[:, :n2], AF.Relu,
                        bias=bz[:, 2:3], scale=sc["A_s"],
                    )
                    nc.scalar.activation(
                        h2i[:, :n2], z1i[:, :n2], AF.Relu,
                        bias=bz[:, 3:4], scale=sc["A_s"],
                    )

                    # ---- layer-2 aggregation + real projection (v1 trick)
                    h2c = sp.tile([D, 512], bf16, tag="h2c")
                    if sc["h2c_on_r"]:
                        nc.vector.scalar_tensor_tensor(
                            h2c[:, :n2], h2r[:, :n2], sc["h2c_scale"],
                            h2i[:, :n2], OP.mult, OP.add,
                        )
                    else:
                        nc.vector.scalar_tensor_tensor(
                            h2c[:, :n2], h2i[:, :n2], sc["h2c_scale"],
                            h2r[:, :n2], OP.mult, OP.add,
                        )
                    u2 = sp.tile([D, 512], f32, tag="u2")
                    nc.vector.tensor_tensor_scan(
                        u2[:, :n2], h2c[:, :n2], h2c[:, :n2], 0.0,
                        OP.add, OP.bypass,
                    )
                    t1 = sp.tile([D, 512], f32, tag="t1")
                    t2 = sp.tile([D, 512], f32, tag="t2")
                    t3 = sp.tile([D, 512], f32, tag="t3")
                    ot = sp.tile([D, 512], bf16, tag="ot")
                    nc.vector.scalar_tensor_tensor(
                        t1[:, :cw], h2i[:, 9 : 9 + cw], sc["r1"],
                        h2r[:, 9 : 9 + cw], OP.mult, OP.add,
                    )
                    nc.vector.scalar_tensor_tensor(
                        t2[:, :cw], u2[:, 17 : 17 + cw], sc["qn"], t1[:, :cw],
                        OP.mult, OP.add,
                    )
                    nc.vector.scalar_tensor_tensor(
                        t3[:, :cw], u2[:, 0:cw], -sc["qn"], t2[:, :cw],
                        OP.mult, OP.add,
                    )
                    nc.scalar.activation(
                        ot[:, :cw], t3[:, :cw], AF.Identity,
                        bias=bz[:, 4:5], scale=sc["c0"],
                    )

                    # ---- node-major output via XBAR block transposes
                    written = 0
                    nb = (cw + 127) // 128
                    for b in range(nb):
                        s0 = min(b * 128, cw - 128)
                        obt = op_.tile([128, 128], bf16)
                        nc.sync.dma_start(
                            obt[:, :], ot[:, s0 : s0 + 128], transpose=True
                        )
                        p0 = written - s0
                        rows = s0 + 128 - written
                        r0 = k * CH2 + written
                        nc.sync.dma_start(
                            out_d[r0 : r0 + rows, :], obt[p0 : p0 + rows, :]
                        )
                        written = s0 + 128

    nc.compile()
    return nc


# ------------------------------------------------------- v1 legacy program
def _fold_weights(ins, w1, w2):
    W1r, W1i = ins["W1r"], ins["W1i"]
    W2c = ins["W2r"] + 1j * ins["W2i"]
    b2c = ins["b2r"] + 1j * ins["b2i"]
    eb1c = ins["eb1r"] + 1j * ins["eb1i"]
    w0_1, wb_1 = w1[0], w1[1:].mean()
    w0_2, wb_2 = w2[0], w2[1:].mean()
    Wa = (w0_1 - wb_1) * W2c
    Wb = wb_1 * W2c
    b2p = b2c + W2c @ eb1c
    c_h2r = (w0_2 - wb_2).real
    c_h2i = -(w0_2 - wb_2).imag
    c_nr = wb_2.real
    c_ni = -wb_2.imag
    c0 = c_h2r
    f32 = np.float32
    wl1 = np.concatenate([W1r.T, W1i.T], axis=1).astype(f32)
    wg = np.concatenate(
        [
            Wa.real.T, -Wa.imag.T, Wb.real.T, -Wb.imag.T,
            Wa.imag.T, Wa.real.T, Wb.imag.T, Wb.real.T,
        ],
        axis=1,
    ).astype(f32)
    biases = np.stack(
        [
            ins["b1r"], ins["b1i"], b2p.real, b2p.imag, ins["eb2r"],
            np.zeros(D), np.zeros(D), np.zeros(D),
        ],
        axis=1,
    ).astype(f32)
    if max(abs(c_nr), abs(c_ni)) == 0.0:
        h2c_on_r, h2c_scale, q = False, 0.0, 0.0
    elif abs(c_nr) >= abs(c_ni):
        h2c_on_r, h2c_scale, q = False, c_ni / c_nr, c_nr
    else:
        h2c_on_r, h2c_scale, q = True, c_nr / c_ni, c_ni
    scalars = dict(
        r1=float(c_h2i / c0), c0=float(c0),
        h2c_on_r=bool(h2c_on_r), h2c_scale=float(h2c_scale), qn=float(q / c0),
    )
    return wl1, wg, biases, scalars


def _build_program(matmul_dtype="float32r", reps=1):
    import concourse.bacc as bacc
    import concourse.mybir as mybir
    import concourse.tile as tile

    f32 = mybir.dt.float32
    mm_dt = getattr(mybir.dt, matmul_dtype)
    AF = mybir.ActivationFunctionType
    OP = mybir.AluOpType

    nc = bacc.Bacc("TRN2", target_bir_lowering=False, debug=False)

    xT = nc.dram_tensor("xT", [D, LH], mm_dt, kind="ExternalInput")
    wl1_d = nc.dram_tensor("wl1", [D, 2 * D], mm_dt, kind="ExternalInput")
    wg_d = nc.dram_tensor("wg", [D, 8 * D], mm_dt, kind="ExternalInput")
    bias_d = nc.dram_tensor("biases", [D, 8], f32, kind="ExternalInput")
    outT = nc.dram_tensor("outT", [D, L], f32, kind="ExternalOutput")

    with tile.TileContext(nc) as tc:
        with (
            tc.tile_pool(name="consts", bufs=1) as cpool,
            tc.tile_pool(name="slab", bufs=1) as slab,
            tc.tile_pool(name="xs", bufs=3) as xs,
            tc.tile_pool(name="ps1", bufs=2, space="PSUM") as ps1,
            tc.tile_pool(name="ps2", bufs=2, space="PSUM") as ps2,
            tc.tile_pool(name="st2", bufs=3) as st2,
            tc.tile_pool(name="outp", bufs=3) as outp,
        ):
            wl1 = cpool.tile([D, 2 * D], mm_dt)
            wg = cpool.tile([D, 8 * D], mm_dt)
            bias = cpool.tile([D, 8], f32)
            nc.sync.dma_start(wl1[:], wl1_d[:])
            nc.sync.dma_start(wg[:], wg_d[:])
            nc.sync.dma_start(bias[:], bias_d[:])

            for _rep in range(reps):
              h1r = slab.tile([D, LH], mm_dt, tag="h1r")
              h1i = slab.tile([D, LH], mm_dt, tag="h1i")

              n1 = (LH + L1_CHUNK - 1) // L1_CHUNK
              for k in range(n1):
                  s = k * L1_CHUNK
                  cw = min(L1_CHUNK, LH - s)
                  xt = xs.tile([D, L1_CHUNK], mm_dt)
                  nc.sync.dma_start(xt[:, :cw], xT[:, s : s + cw])
                  pr = ps1.tile([D, L1_CHUNK], f32, tag="ps1r")
                  pi = ps1.tile([D, L1_CHUNK], f32, tag="ps1i")
                  nc.tensor.matmul(
                      pr[:, :cw], wl1[:, 0:D], xt[:, :cw], start=True, stop=True
                  )
                  nc.tensor.matmul(
                      pi[:, :cw], wl1[:, D : 2 * D], xt[:, :cw],
                      start=True, stop=True,
                  )
                  nc.scalar.activation(
                      h1r[:, s : s + cw], pr[:, :cw], AF.Relu, bias=bias[:, 0:1]
                  )
                  nc.scalar.activation(
                      h1i[:, s : s + cw], pi[:, :cw], AF.Relu, bias=bias[:, 1:2]
                  )

              n2 = (L + CHUNK - 1) // CHUNK
              for k in range(n2):
                  a = HL + k * CHUNK
                  cw = min(CHUNK, L - k * CHUNK)
                  w2n = cw + 17 + ((cw + 17) % 2)
                  w1s, w1n = a - 18, w2n + 17
                  w2s = a - 9

                  dve_view = (
                      (lambda ap: ap.bitcast(f32))
                      if matmul_dtype == "float32r"
                      else (lambda ap: ap)
                  )
                  p1r = st2.tile([D, CHUNK + 36], f32, tag="p1r")
                  p1i = st2.tile([D, CHUNK + 36], f32, tag="p1i")
                  nc.vector.tensor_tensor_scan(
                      p1r[:, :w1n],
                      dve_view(h1r[:, w1s : w1s + w1n]),
                      dve_view(h1r[:, w1s : w1s + w1n]),
                      0.0, OP.add, OP.bypass,
                  )
                  nc.vector.tensor_tensor_scan(
                      p1i[:, :w1n],
                      dve_view(h1i[:, w1s : w1s + w1n]),
                      dve_view(h1i[:, w1s : w1s + w1n]),
                      0.0, OP.add, OP.bypass,
                  )
                  ns1r = st2.tile([D, CHUNK + 18], mm_dt, tag="ns1r")
                  ns1i = st2.tile([D, CHUNK + 18], mm_dt, tag="ns1i")
                  nc.vector.scalar_tensor_tensor(
                      ns1r[:, :w2n], p1r[:, 17 : 17 + w2n], 1.0, p1r[:, 0:w2n],
                      OP.mult, OP.subtract,
                  )
                  nc.vector.scalar_tensor_tensor(
                      ns1i[:, :w2n], p1i[:, 17 : 17 + w2n], 1.0, p1i[:, 0:w2n],
                      OP.mult, OP.subtract,
                  )

                  pgr = ps2.tile([D, CHUNK + 18], f32, tag="ps2r")
                  pgi = ps2.tile([D, CHUNK + 18], f32, tag="ps2i")
                  rhs_list = [
                      h1r[:, w2s : w2s + w2n],
                      h1i[:, w2s : w2s + w2n],
                      ns1r[:, :w2n],
                      ns1i[:, :w2n],
                  ]
                  for comp, ptile in ((0, pgr), (1, pgi)):
                      for t_i, rhs in enumerate(rhs_list):
                          wcol = (comp * 4 + t_i) * D
                          nc.tensor.matmul(
                              ptile[:, :w2n], wg[:, wcol : wcol + D], rhs,
                              start=(t_i == 0), stop=(t_i == 3),
                          )

                  h2r = st2.tile([D, CHUNK + 18], f32, tag="h2r")
                  h2i = st2.tile([D, CHUNK + 18], f32, tag="h2i")
                  nc.scalar.activation(
                      h2r[:, :w2n], pgr[:, :w2n], AF.Relu, bias=bias[:, 2:3]
                  )
                  nc.scalar.activation(
                      h2i[:, :w2n], pgi[:, :w2n], AF.Relu, bias=bias[:, 3:4]
                  )

                  sc = _build_program.scalars
                  h2c = st2.tile([D, CHUNK + 18], f32, tag="h2c")
                  if sc["h2c_on_r"]:
                      nc.vector.scalar_tensor_tensor(
                          h2c[:, :w2n], h2r[:, :w2n], sc["h2c_scale"],
                          h2i[:, :w2n], OP.mult, OP.add,
                      )
                  else:
                      nc.vector.scalar_tensor_tensor(
                          h2c[:, :w2n], h2i[:, :w2n], sc["h2c_scale"],
                          h2r[:, :w2n], OP.mult, OP.add,
                      )
                  u = st2.tile([D, CHUNK + 18], f32, tag="u")
                  nc.vector.tensor_tensor_scan(
                      u[:, :w2n], h2c[:, :w2n], h2c[:, :w2n], 0.0,
                      OP.add, OP.bypass,
                  )
                  t1 = st2.tile([D, CHUNK], f32, tag="t1")
                  t2 = st2.tile([D, CHUNK], f32, tag="t2")
                  t3 = st2.tile([D, CHUNK], f32, tag="t3")
                  ot = outp.tile([D, CHUNK], f32)
                  nc.vector.scalar_tensor_tensor(
                      t1[:, :cw], h2i[:, 9 : 9 + cw], sc["r1"],
                      h2r[:, 9 : 9 + cw], OP.mult, OP.add,
                  )
                  nc.vector.scalar_tensor_tensor(
                      t2[:, :cw], u[:, 17 : 17 + cw], sc["qn"], t1[:, :cw],
                      OP.mult, OP.add,
                  )
                  nc.vector.scalar_tensor_tensor(
                      t3[:, :cw], u[:, 0:cw], -sc["qn"], t2[:, :cw],
                      OP.mult, OP.add,
                  )
                  nc.vector.tensor_scalar(
                      ot[:, :cw], t3[:, :cw], sc["c0"], bias[:, 4:5],
                      OP.mult, OP.add,
                  )
                  nc.sync.dma_start(outT[:, k * CHUNK : k * CHUNK + cw], ot[:, :cw])

    nc.compile()
    return nc


_MM_DTYPE = "float32r"


# ------------------------------------------------------- cached PJRT runner
_EXECS = {}   # id(nc) -> (sharded_fn, in_names, out_names, out_avals)
_DEVCACHE = {}  # name -> (bytes_key, device_array)
_ZEROS = {}   # (shape, dtype-str) -> device array


def _get_executable(nc):
    global _EXECS
    key = id(nc)
    if key in _EXECS:
        return _EXECS[key]
    import jax
    import numpy as _np
    from jax.sharding import Mesh, PartitionSpec
    from jax.experimental.shard_map import shard_map

    import concourse.mybir as mybir
    from concourse import bass2jax

    bass2jax.install_neuronx_cc_hook()

    partition_name = (
        nc.partition_id_tensor.name if nc.partition_id_tensor else None
    )
    in_names, out_names, out_avals = [], [], []
    for alloc in nc.m.functions[0].allocations:
        if not isinstance(alloc, mybir.MemoryLocationSet):
            continue
        name = alloc.memorylocations[0].name
        if alloc.kind == "ExternalInput":
            if name != partition_name:
                in_names.append(name)
        elif alloc.kind == "ExternalOutput":
            out_names.append(name)
            out_avals.append(
                jax.core.ShapedArray(
                    tuple(alloc.tensor_shape), mybir.dt.np(alloc.dtype)
                )
            )
    all_names = in_names + out_names
    if partition_name is not None:
        all_names = all_names + [partition_name]

    def _body(*args):
        operands = list(args)
        if partition_name is not None:
            operands.append(bass2jax.partition_id_tensor())
        outs = bass2jax._bass_exec_p.bind(
            *operands,
            out_avals=tuple(out_avals),
            in_names=tuple(all_names),
            out_names=tuple(out_names),
            lowering_input_output_aliases=(),
            sim_require_finite=True,
            sim_require_nnan=True,
            nc=nc,
        )
        return tuple(outs)

    devices = jax.devices()[:NCORES]
    mesh = Mesh(_np.asarray(devices), ("core",))
    n_in = len(in_names) + len(out_names)
    sharded = jax.jit(
        shard_map(
            _body,
            mesh=mesh,
            in_specs=(PartitionSpec("core"),) * n_in,
            out_specs=(PartitionSpec("core"),) * len(out_names),
            check_rep=False,
        ),
        keep_unused=True,
    )
    _EXECS[key] = (sharded, in_names, out_names, out_avals)
    return _EXECS[key]


def _sharding():
    import jax
    from jax.sharding import Mesh, NamedSharding, PartitionSpec

    mesh = Mesh(np.asarray(jax.devices()[:NCORES]), ("core",))
    return NamedSharding(mesh, PartitionSpec("core"))


def _dev_cached(name, arr):
    """Device-put arr (replicated-per-core concat) unless bytes unchanged."""
    import jax

    key = arr.tobytes()
    hit = _DEVCACHE.get(name)
    if hit is not None and hit[0] == key:
        return hit[1]
    dev = jax.device_put(arr, _sharding())
    jax.block_until_ready(dev)
    _DEVCACHE[name] = (key, dev)
    return dev


def _zeros_for(aval):
    import jax
    import jax.numpy as jnp

    key = (tuple(aval.shape), str(aval.dtype))
    if key not in _ZEROS:
        z = jax.device_put(
            jnp.zeros((NCORES * aval.shape[0], *aval.shape[1:]), aval.dtype),
            _sharding(),
        )
        jax.block_until_ready(z)
        _ZEROS[key] = z
    return _ZEROS[key]


def _execute(nc, arrays):
    """Run nc with named concat input arrays; returns dict of host outputs."""
    sharded, in_names, out_names, out_avals = _get_executable(nc)
    args = []
    for name in in_names:
        a = arrays[name]
        if name in ("wz", "bz", "wl1", "wg", "biases"):
            args.append(_dev_cached(name, a))
        else:
            args.append(a)
    zeros = [_zeros_for(a) for a in out_avals]
    out_arrs = sharded(*args, *zeros)
    return {
        name: np.asarray(out_arrs[i]).reshape(NCORES, *out_avals[i].shape)
        for i, name in enumerate(out_names)
    }


# ---------------------------------------------------------------- v2 runner
def _get_program2(scalars):
    global _P2, _P2_scalars
    if _P2 is None or _P2_scalars != scalars:
        _P2 = _build_program2(scalars)
        _P2_scalars = dict(scalars)
    return _P2


def _prepare2(ins):
    import ml_dtypes

    bf16 = np.dtype(ml_dtypes.bfloat16)
    w1 = _evolution_row(DEG, float(ins["t1r"]), float(ins["t1i"]))
    w2 = _evolution_row(DEG, float(ins["t2r"]), float(ins["t2i"]))
    wz, bz, scalars = _fold2(ins, w1, w2)

    x = ins["x"]
    xbf = x.astype(bf16)
    xs_all = np.empty((NCORES * LH2, D), bf16)
    for c in range(NCORES):
        v = xs_all[c * LH2 : (c + 1) * LH2]
        lo = c * L
        v[HL2 : HL2 + L] = xbf[lo : lo + L]
        if lo >= HL2:
            v[:HL2] = xbf[lo - HL2 : lo]
        else:
            v[:HL2] = xbf[N - HL2 :]
        hi = lo + L
        if hi + HR2 <= N:
            v[HL2 + L :] = xbf[hi : hi + HR2]
        else:
            v[HL2 + L :] = xbf[: HR2]
    arrays = {
        "xs": xs_all,
        "wz": np.broadcast_to(wz, (NCORES, *wz.shape)).reshape(NCORES * D, -1),
        "bz": np.broadcast_to(bz, (NCORES, *bz.shape)).reshape(NCORES * D, -1),
    }
    arrays = {k: np.ascontiguousarray(a) for k, a in arrays.items()}
    return arrays, scalars


def _run_v2(ins):
    arrays, scalars = _prepare2(ins)
    nc = _get_program2(scalars)
    outs = _execute(nc, arrays)
    return np.ascontiguousarray(
        outs["outN"].reshape(N, D).astype(np.float32)
    )


# ------------------------------------------------------------- legacy runner
def _get_program(scalars):
    global _PROGRAM
    _build_program.scalars = scalars
    if _PROGRAM is None:
        _PROGRAM = _build_program(_MM_DTYPE)
    return _PROGRAM


def _reset_program(mm_dtype):
    global _MM_DTYPE, _PROGRAM
    _MM_DTYPE = mm_dtype
    _PROGRAM = None


def _prepare(ins):
    w1 = _evolution_row(DEG, float(ins["t1r"]), float(ins["t1i"]))
    w2 = _evolution_row(DEG, float(ins["t2r"]), float(ins["t2i"]))
    wl1, wg, biases, scalars = _fold_weights(ins, w1, w2)
    _get_program(scalars)

    x = ins["x"].astype(np.float32, copy=False)
    idx = np.arange(-HL, L + HR)
    xs_parts = []
    for c in range(NCORES):
        rows = (c * L + idx) % N
        xs_parts.append(np.ascontiguousarray(x[rows].T))
    arrays = {
        "xT": np.concatenate(xs_parts, axis=0),
        "wl1": np.broadcast_to(wl1, (NCORES, *wl1.shape)).reshape(NCORES * D, -1),
        "wg": np.broadcast_to(wg, (NCORES, *wg.shape)).reshape(NCORES * D, -1),
        "biases": np.broadcast_to(biases, (NCORES, *biases.shape)).reshape(
            NCORES * D, -1
        ),
    }
    arrays = {k: np.ascontiguousarray(a) for k, a in arrays.items()}
    return arrays


def _run_legacy(ins):
    arrays = _prepare(ins)
    outs = _execute(_PROGRAM, arrays)
    out = np.empty((N, D), np.float32)
    for c in range(NCORES):
        out[c * L : (c + 1) * L] = outs["outT"][c].T
    return out


# ---------------------------------------------------------------- entrypoint
def _run(ins, trace=False):
    ins = {k: np.asarray(v) for k, v in ins.items()}
    if not _is_circulant(ins["edge_index"]):
        return _fallback_numpy(ins), None
    try:
        return _run_v2(ins), None
    except _Degenerate:
        return _fallback_numpy(ins), None
    except Exception:
        import traceback

        traceback.print_exc()
    try:
        return _run_legacy(ins), None
    except Exception:
        if _MM_DTYPE == "float32":
            raise
        _reset_program("float32")
        return _run_legacy(ins), None


def kernel(**inputs):
    out, _ = _run(inputs)
    return out


# revision 5
# speedup vs baseline: 856.0750x; 1.1477x over previous
"""Trainium2 Bass kernel for nn_ComplexUnitaryGCN (2-layer complex unitary GCN,
circulant 16-regular graph, N=100000 nodes, D=128 dims, 8 NeuronCores).

v2 pipeline (primary):
  - Nodes sharded across 8 cores with replicated halos; per-core input is a
    host-built [12544, 128] bf16 slab (3 contiguous memcpys per core).
  - On device, the slab is transposed feature-major by the DMA XBAR
    (16x128-tile transpose DMA, bf16) straight into SBUF.
  - Layer 1 (h1 = crelu(W1 x + b1)) is pointwise in nodes: bf16 GEMM chunks.
  - Key restructure vs v1: the layer-1 star aggregation commutes with the
    layer-2 GEMM, so compute C = W2c @ h1 FIRST (4 bf16 matmuls / chunk
    instead of 8) and apply  alpha*C + beta*window17(C)  elementwise.
    Window sums come from one prefix scan per component (DVE/Pool), with
    combos normalized by the dominant scalar component for conditioning.
  - The layer-2 aggregation + real projection is the v1 scan trick.
  - Elementwise work is split across DVE / GpSimd / ScalarE; output is
    transposed back node-major by XBAR SBUF->SBUF block transposes and
    leaves as bf16 [12500, 128] per core.
  - Host: cached circulant check, cached evolution rows, cached device
    weights + output containers; only x (25MB bf16) moves per call.
Falls back to the v1 f32r program, then f32, then a numpy reference.
"""

import numpy as np

# ---------------------------------------------------------------- constants
N = 100000
D = 128
NCORES = 8
L = N // NCORES           # 12500 nodes per core
DEG = 16

# ---- v2 geometry
HL2, HR2 = 24, 20         # halos; LH2 must be a multiple of 16 for the XBAR
LH2 = L + HL2 + HR2       # 12544 slab rows
L1C = 1024                # layer-1 chunk (two 512 PSUM halves per component)
CH2 = 464                 # stage-2 output chunk; n1 = CH2+36 = 500 <= 512
NCH2 = 27                 # 26*464 + 436 = 12500

# ---- v1 geometry (legacy fallback)
HL, HR = 18, 18
LH = L + HL + HR          # 12534
CHUNK = 492
L1_CHUNK = 512

_PROGRAM = None           # legacy compiled program
_P2 = None                # v2 compiled program
_P2_scalars = None        # scalars baked into _P2


# ------------------------------------------------------------- host helpers
def _evolution_row_impl(deg, tr, ti):
    try:
        import jax

        cpu = jax.devices("cpu")[0]
        with jax.default_device(cpu):
            import jax.numpy as jnp

            n = deg + 1
            A = jnp.zeros((n, n), jnp.complex64).at[0, 1:].set(1.0).at[1:, 0].set(1.0)
            t = (jnp.float32(tr) + 1j * jnp.float32(ti)).astype(jnp.complex64)
            G = jax.scipy.linalg.expm(-1j * A * t)
            s = jnp.sqrt(jnp.max(jnp.linalg.eigvalsh(G @ G.conj().T))).astype(
                jnp.complex64
            )
            Lt = G / s
            Rt = jnp.sqrt(jnp.eye(n, dtype=jnp.complex64) - Lt @ (G.conj().T / s))
            return np.asarray(Lt[0] + Rt[0])
    except Exception:
        n = deg + 1
        A = np.zeros((n, n), np.float64)
        A[0, 1:] = 1.0
        A[1:, 0] = 1.0
        t = complex(tr, ti)
        evals, evecs = np.linalg.eigh(A)
        G = (evecs * np.exp(-1j * evals * t)) @ evecs.T
        s = np.sqrt(np.max(np.linalg.eigvalsh(G @ G.conj().T)))
        Lt = G / s
        Rt = np.sqrt(np.eye(n) - Lt @ (G.conj().T / s))
        return (Lt[0] + Rt[0]).astype(np.complex64)


_EVO_CACHE = {}


def _evolution_row(deg, tr, ti):
    key = (deg, float(tr), float(ti))
    if key not in _EVO_CACHE:
        _EVO_CACHE[key] = _evolution_row_impl(deg, tr, ti)
    return _EVO_CACHE[key]


_CIRC_EXPECT = None


def _is_circulant(edge_index):
    """Check edge_index matches the reference's circulant construction."""
    global _CIRC_EXPECT
    if edge_index.shape != (2, N * DEG // 2):
        return False
    if _CIRC_EXPECT is None:
        K = DEG // 2
        i = np.arange(N, dtype=edge_index.dtype)
        src = np.repeat(i, K)
        dst = ((i[:, None] + np.arange(1, K + 1, dtype=edge_index.dtype)[None, :])
               % N).reshape(-1)
        _CIRC_EXPECT = np.stack([src, dst]).astype(np.int32)
    return bool(np.array_equal(edge_index, _CIRC_EXPECT))


def _fallback_numpy(ins):
    """Exact reference semantics on host (any edge_index). Slow but correct."""
    x = ins["x"]
    edge_index = ins["edge_index"]
    src, dst = edge_index[0], edge_index[1]
    nodes = np.concatenate([src, dst])
    nbr = np.concatenate([dst, src])
    order = np.lexsort((nbr, nodes))
    deg = nodes.shape[0] // N
    nbrs = nbr[order].reshape(N, deg)
    h = x.astype(np.complex64)

    def crelu(z):
        return (np.maximum(z.real, 0) + 1j * np.maximum(z.imag, 0)).astype(
            np.complex64
        )

    for l in ("1", "2"):
        W = (ins[f"W{l}r"] + 1j * ins[f"W{l}i"]).astype(np.complex64)
        b = (ins[f"b{l}r"] + 1j * ins[f"b{l}i"]).astype(np.complex64)
        h = crelu(h @ W.T + b)
        w = _evolution_row(deg, float(ins[f"t{l}r"]), float(ins[f"t{l}i"]))
        out = w[0] * h
        for k in range(deg):
            out = out + w[1 + k] * h[nbrs[:, k]]
        h = (out + (ins[f"eb{l}r"] + 1j * ins[f"eb{l}i"]).astype(np.complex64))
        h = h.astype(np.complex64)
    return np.ascontiguousarray(h.real.astype(np.float32))


class _Degenerate(Exception):
    pass


# ----------------------------------------------------------- v2 weight fold
def _fold2(ins, w1, w2):
    """Device layouts + baked scalars for the v2 program."""
    import ml_dtypes

    bf16 = np.dtype(ml_dtypes.bfloat16)
    W1r, W1i = ins["W1r"], ins["W1i"]
    W2r, W2i = ins["W2r"], ins["W2i"]
    W2c = W2r + 1j * W2i
    b2c = ins["b2r"] + 1j * ins["b2i"]
    eb1c = ins["eb1r"] + 1j * ins["eb1i"]
    b2p = b2c + W2c @ eb1c

    alpha = complex(w1[0] - w1[1:].mean())
    beta = complex(w1[1:].mean())
    a2 = complex(w2[0] - w2[1:].mean())
    b2s = complex(w2[1:].mean())

    # wz cols: W1rT | W1iT | W2rT | W2iT | -W2iT   (lhsT layout [in, out])
    wz = np.concatenate(
        [W1r.T, W1i.T, W2r.T, W2i.T, -W2i.T], axis=1
    ).astype(bf16)                                               # [128, 640]
    biases = np.stack(
        [
            ins["b1r"], ins["b1i"], b2p.real.astype(np.float32),
            b2p.imag.astype(np.float32), ins["eb2r"],
            np.zeros(D, np.float32), np.zeros(D, np.float32),
            np.zeros(D, np.float32),
        ],
        axis=1,
    ).astype(np.float32)                                         # [128, 8]

    mag = abs(alpha) + abs(beta)
    # window-17 combos normalized by the dominant component of beta / alpha
    if abs(beta.real) >= abs(beta.imag):
        B_s = beta.real
        s_ur, ur_form = (-beta.imag / B_s if B_s else 0.0), "cis_first_add"
        s_ui, ui_form = (beta.imag / B_s if B_s else 0.0), "cr_first_add"
    else:
        B_s = beta.imag
        s_ur, ur_form = beta.real / B_s, "cr_first_sub"
        s_ui, ui_form = beta.real / B_s, "cis_first_add"
    if abs(alpha.real) >= abs(alpha.imag):
        A_s = alpha.real
        if abs(A_s) < 1e-12 * (mag + 1e-30):
            raise _Degenerate("alpha ~ 0")
        s_vr, vr_form = -alpha.imag / A_s, "cis_first_add"
        s_vi, vi_form = alpha.imag / A_s, "cr_first_add"
    else:
        A_s = alpha.imag
        s_vr, vr_form = alpha.real / A_s, "cr_first_sub"
        s_vi, vi_form = alpha.real / A_s, "cis_first_add"
    gam = (B_s / A_s) if B_s else 0.0

    c_h2r, c_h2i = a2.real, -a2.imag
    c_nr, c_ni = b2s.real, -b2s.imag
    c0 = c_h2r
    if abs(c0) < 1e-12 * (abs(a2) + abs(b2s) + 1e-30):
        raise _Degenerate("c0 ~ 0")
    r1 = c_h2i / c0
    if max(abs(c_nr), abs(c_ni)) == 0.0:
        h2c_on_r, h2c_scale, q = False, 0.0, 0.0
    elif abs(c_nr) >= abs(c_ni):
        h2c_on_r, h2c_scale, q = False, c_ni / c_nr, c_nr
    else:
        h2c_on_r, h2c_scale, q = True, c_nr / c_ni, c_ni
    scalars = dict(
        s_ur=float(s_ur), ur_form=ur_form, s_ui=float(s_ui), ui_form=ui_form,
        s_vr=float(s_vr), vr_form=vr_form, s_vi=float(s_vi), vi_form=vi_form,
        gam=float(gam), A_s=float(A_s),
        r1=float(r1), c0=float(c0),
        h2c_on_r=bool(h2c_on_r), h2c_scale=float(h2c_scale),
        qn=float(q / c0),
    )
    return wz, biases, scalars


# ------------------------------------------------------------ v2 device program
def _build_program2(scalars, reps=1):
    import concourse.bacc as bacc
    import concourse.mybir as mybir
    import concourse.tile as tile

    f32 = mybir.dt.float32
    bf16 = mybir.dt.bfloat16
    AF = mybir.ActivationFunctionType
    OP = mybir.AluOpType
    sc = scalars

    nc = bacc.Bacc("TRN2", target_bir_lowering=False, debug=False)

    xs_d = nc.dram_tensor("xs", [LH2, D], bf16, kind="ExternalInput")
    wz_d = nc.dram_tensor("wz", [D, 5 * D], bf16, kind="ExternalInput")
    bz_d = nc.dram_tensor("bz", [D, 8], f32, kind="ExternalInput")
    out_d = nc.dram_tensor("outN", [L, D], bf16, kind="ExternalOutput")

    def combo(eng, out_ap, form, s, cr_ap, cis_ap):
        """out = alpha-normalized linear combo of (Cr, Ci)."""
        if form == "cis_first_add":      # (cis * s) + Cr
            eng.scalar_tensor_tensor(out_ap, cis_ap, s, cr_ap, OP.mult, OP.add)
        elif form == "cr_first_add":     # (Cr * s) + cis
            eng.scalar_tensor_tensor(out_ap, cr_ap, s, cis_ap, OP.mult, OP.add)
        elif form == "cr_first_sub":     # (Cr * s) - cis
            eng.scalar_tensor_tensor(out_ap, cr_ap, s, cis_ap, OP.mult, OP.subtract)
        else:
            raise ValueError(form)

    with tile.TileContext(nc) as tc:
        with (
            tc.tile_pool(name="consts", bufs=1) as cpool,
            tc.tile_pool(name="slab", bufs=1) as slab,
            tc.tile_pool(name="xp", bufs=2) as xp,
            tc.tile_pool(name="l1p", bufs=1, space="PSUM") as l1p,
            tc.tile_pool(name="cps", bufs=2, space="PSUM") as cps,
            tc.tile_pool(name="sp", bufs=3) as sp,
            tc.tile_pool(name="op", bufs=3) as op_,
        ):
            wz = cpool.tile([D, 5 * D], bf16)
            bz = cpool.tile([D, 8], f32)
            nc.sync.dma_start(wz[:], wz_d[:])
            nc.sync.dma_start(bz[:], bz_d[:])

            for _rep in range(reps):
                h1r = slab.tile([D, LH2], bf16, tag="h1r")
                h1i = slab.tile([D, LH2], bf16, tag="h1i")

                # ---- layer 1 over the whole slab (XBAR-transposed loads)
                nl1 = (LH2 + L1C - 1) // L1C
                for j in range(nl1):
                    s0 = j * L1C
                    cols = min(L1C, LH2 - s0)
                    xt = xp.tile([D, L1C], bf16)
                    nc.sync.dma_start(
                        xt[:, :cols], xs_d[s0 : s0 + cols, :], transpose=True
                    )
                    for hh in range(0, cols, 512):
                        w = min(512, cols - hh)
                        tg = hh // 512
                        pr = l1p.tile([D, 512], f32, tag=f"pr{tg}")
                        pi = l1p.tile([D, 512], f32, tag=f"pi{tg}")
                        nc.tensor.matmul(
                            pr[:, :w], wz[:, 0:D], xt[:, hh : hh + w],
                            start=True, stop=True,
                        )
                        nc.tensor.matmul(
                            pi[:, :w], wz[:, D : 2 * D], xt[:, hh : hh + w],
                            start=True, stop=True,
                        )
                        nc.scalar.activation(
                            h1r[:, s0 + hh : s0 + hh + w], pr[:, :w],
                            AF.Relu, bias=bz[:, 0:1],
                        )
                        nc.scalar.activation(
                            h1i[:, s0 + hh : s0 + hh + w], pi[:, :w],
                            AF.Relu, bias=bz[:, 1:2],
                        )

                # ---- stage 2: per out-chunk [a, a+cw) in slab coords
                for k in range(NCH2):
                    a = HL2 + k * CH2
                    cw = min(CH2, L - k * CH2)
                    o1 = a - 18
                    n1 = cw + 36
                    n2 = cw + 18
                    # C = W2c @ h1 over [o1, o1+n1)
                    cr = cps.tile([D, 512], f32, tag="cr")
                    ci = cps.tile([D, 512], f32, tag="ci")
                    r_sl = h1r[:, o1 : o1 + n1]
                    i_sl = h1i[:, o1 : o1 + n1]
                    nc.tensor.matmul(
                        cr[:, :n1], wz[:, 2 * D : 3 * D], r_sl,
                        start=True, stop=False,
                    )
                    nc.tensor.matmul(
                        cr[:, :n1], wz[:, 4 * D : 5 * D], i_sl,
                        start=False, stop=True,
                    )
                    nc.tensor.matmul(
                        ci[:, :n1], wz[:, 3 * D : 4 * D], r_sl,
                        start=True, stop=False,
                    )
                    nc.tensor.matmul(
                        ci[:, :n1], wz[:, 2 * D : 3 * D], i_sl,
                        start=False, stop=True,
                    )
                    # GpSimd cannot read PSUM: stage both C components to SBUF
                    crs = sp.tile([D, 512], bf16, tag="crs")
                    cis = sp.tile([D, 512], bf16, tag="cis")
                    nc.scalar.copy(crs[:, :n1], cr[:, :n1])
                    nc.scalar.copy(cis[:, :n1], ci[:, :n1])

                    u1r = sp.tile([D, 512], bf16, tag="u1r")
                    u1i = sp.tile([D, 512], bf16, tag="u1i")
                    combo(nc.vector, u1r[:, :n1], sc["ur_form"], sc["s_ur"],
                          crs[:, :n1], cis[:, :n1])
                    combo(nc.vector, u1i[:, :n1], sc["ui_form"], sc["s_ui"],
                          crs[:, :n1], cis[:, :n1])
                    p1r = sp.tile([D, 512], f32, tag="p1r")
                    p1i = sp.tile([D, 512], f32, tag="p1i")
                    nc.vector.tensor_tensor_scan(
                        p1r[:, :n1], u1r[:, :n1], u1r[:, :n1], 0.0,
                        OP.add, OP.bypass,
                    )
                    nc.vector.tensor_tensor_scan(
                        p1i[:, :n1], u1i[:, :n1], u1i[:, :n1], 0.0,
                        OP.add, OP.bypass,
                    )
                    d_r = sp.tile([D, 512], bf16, tag="d_r")
                    d_i = sp.tile([D, 512], bf16, tag="d_i")
                    nc.vector.scalar_tensor_tensor(
                        d_r[:, :n2], p1r[:, 17 : 17 + n2], 1.0, p1r[:, 0:n2],
                        OP.mult, OP.subtract,
                    )
                    nc.vector.scalar_tensor_tensor(
                        d_i[:, :n2], p1i[:, 17 : 17 + n2], 1.0, p1i[:, 0:n2],
                        OP.mult, OP.subtract,
                    )
                    v1r = sp.tile([D, 512], bf16, tag="v1r")
                    v1i = sp.tile([D, 512], bf16, tag="v1i")
                    combo(nc.vector, v1r[:, :n2], sc["vr_form"], sc["s_vr"],
                          crs[:, 9 : 9 + n2], cis[:, 9 : 9 + n2])
                    combo(nc.vector, v1i[:, :n2], sc["vi_form"], sc["s_vi"],
                          crs[:, 9 : 9 + n2], cis[:, 9 : 9 + n2])
                    z1r = sp.tile([D, 512], bf16, tag="z1r")
                    z1i = sp.tile([D, 512], bf16, tag="z1i")
                    nc.vector.scalar_tensor_tensor(
                        z1r[:, :n2], d_r[:, :n2], sc["gam"], v1r[:, :n2],
                        OP.mult, OP.add,
                    )
                    nc.vector.scalar_tensor_tensor(
                        z1i[:, :n2], d_i[:, :n2], sc["gam"], v1i[:, :n2],
                        OP.mult, OP.add,
                    )
                    h2r = sp.tile([D, 512], bf16, tag="h2r")
                    h2i = sp.tile([D, 512], bf16, tag="h2i")
                    nc.scalar.activation(
                        h2r[:, :n2], z1r[:, :n2], AF.Relu,
                        bias=bz[:, 2:3], scale=sc["A_s"],
                    )
                    nc.scalar.activation(
                        h2i[:, :n2], z1i[:, :n2], AF.Relu,
                        bias=bz[:, 3:4], scale=sc["A_s"],
                    )

                    # ---- layer-2 aggregation + real projection (v1 trick)
                    h2c = sp.tile([D, 512], bf16, tag="h2c")
                    if sc["h2c_on_r"]:
                        nc.vector.scalar_tensor_tensor(
                            h2c[:, :n2], h2r[:, :n2], sc["h2c_scale"],
                            h2i[:, :n2], OP.mult, OP.add,
                        )
                    else:
                        nc.vector.scalar_tensor_tensor(
                            h2c[:, :n2], h2i[:, :n2], sc["h2c_scale"],
                            h2r[:, :n2], OP.mult, OP.add,
                        )
                    u2 = sp.tile([D, 512], f32, tag="u2")
                    nc.vector.tensor_tensor_scan(
                        u2[:, :n2], h2c[:, :n2], h2c[:, :n2], 0.0,
                        OP.add, OP.bypass,
                    )
                    t1 = sp.tile([D, 512], f32, tag="t1")
                    t2 = sp.tile([D, 512], f32, tag="t2")
                    t3 = sp.tile([D, 512], f32, tag="t3")
                    ot = sp.tile([D, 512], bf16, tag="ot")
                    nc.vector.scalar_tensor_tensor(
                        t1[:, :cw], h2i[:, 9 : 9 + cw], sc["r1"],
                        h2r[:, 9 : 9 + cw], OP.mult, OP.add,
                    )
                    nc.vector.scalar_tensor_tensor(
                        t2[:, :cw], u2[:, 17 : 17 + cw], sc["qn"], t1[:, :cw],
                        OP.mult, OP.add,
                    )
                    nc.vector.scalar_tensor_tensor(
                        t3[:, :cw], u2[:, 0:cw], -sc["qn"], t2[:, :cw],
                        OP.mult, OP.add,
                    )
                    nc.scalar.activation(
                        ot[:, :cw], t3[:, :cw], AF.Identity,
                        bias=bz[:, 4:5], scale=sc["c0"],
                    )

                    # ---- node-major output via XBAR block transposes
                    written = 0
                    nb = (cw + 127) // 128
                    for b in range(nb):
                        s0 = min(b * 128, cw - 128)
                        obt = op_.tile([128, 128], bf16)
                        nc.sync.dma_start(
                            obt[:, :], ot[:, s0 : s0 + 128], transpose=True
                        )
                        p0 = written - s0
                        rows = s0 + 128 - written
                        r0 = k * CH2 + written
                        nc.sync.dma_start(
                            out_d[r0 : r0 + rows, :], obt[p0 : p0 + rows, :]
                        )
                        written = s0 + 128

    nc.compile()
    return nc


# ------------------------------------------------------- v1 legacy program
def _fold_weights(ins, w1, w2):
    W1r, W1i = ins["W1r"], ins["W1i"]
    W2c = ins["W2r"] + 1j * ins["W2i"]
    b2c = ins["b2r"] + 1j * ins["b2i"]
    eb1c = ins["eb1r"] + 1j * ins["eb1i"]
    w0_1, wb_1 = w1[0], w1[1:].mean()
    w0_2, wb_2 = w2[0], w2[1:].mean()
    Wa = (w0_1 - wb_1) * W2c
    Wb = wb_1 * W2c
    b2p = b2c + W2c @ eb1c
    c_h2r = (w0_2 - wb_2).real
    c_h2i = -(w0_2 - wb_2).imag
    c_nr = wb_2.real
    c_ni = -wb_2.imag
    c0 = c_h2r
    f32 = np.float32
    wl1 = np.concatenate([W1r.T, W1i.T], axis=1).astype(f32)
    wg = np.concatenate(
        [
            Wa.real.T, -Wa.imag.T, Wb.real.T, -Wb.imag.T,
            Wa.imag.T, Wa.real.T, Wb.imag.T, Wb.real.T,
        ],
        axis=1,
    ).astype(f32)
    biases = np.stack(
        [
            ins["b1r"], ins["b1i"], b2p.real, b2p.imag, ins["eb2r"],
            np.zeros(D), np.zeros(D), np.zeros(D),
        ],
        axis=1,
    ).astype(f32)
    if max(abs(c_nr), abs(c_ni)) == 0.0:
        h2c_on_r, h2c_scale, q = False, 0.0, 0.0
    elif abs(c_nr) >= abs(c_ni):
        h2c_on_r, h2c_scale, q = False, c_ni / c_nr, c_nr
    else:
        h2c_on_r, h2c_scale, q = True, c_nr / c_ni, c_ni
    scalars = dict(
        r1=float(c_h2i / c0), c0=float(c0),
        h2c_on_r=bool(h2c_on_r), h2c_scale=float(h2c_scale), qn=float(q / c0),
    )
    return wl1, wg, biases, scalars


def _build_program(matmul_dtype="float32r", reps=1):
    import concourse.bacc as bacc
    import concourse.mybir as mybir
    import concourse.tile as tile

    f32 = mybir.dt.float32
    mm_dt = getattr(mybir.dt, matmul_dtype)
    AF = mybir.ActivationFunctionType
    OP = mybir.AluOpType

    nc = bacc.Bacc("TRN2", target_bir_lowering=False, debug=False)

    xT = nc.dram_tensor("xT", [D, LH], mm_dt, kind="ExternalInput")
    wl1_d = nc.dram_tensor("wl1", [D, 2 * D], mm_dt, kind="ExternalInput")
    wg_d = nc.dram_tensor("wg", [D, 8 * D], mm_dt, kind="ExternalInput")
    bias_d = nc.dram_tensor("biases", [D, 8], f32, kind="ExternalInput")
    outT = nc.dram_tensor("outT", [D, L], f32, kind="ExternalOutput")

    with tile.TileContext(nc) as tc:
        with (
            tc.tile_pool(name="consts", bufs=1) as cpool,
            tc.tile_pool(name="slab", bufs=1) as slab,
            tc.tile_pool(name="xs", bufs=3) as xs,
            tc.tile_pool(name="ps1", bufs=2, space="PSUM") as ps1,
            tc.tile_pool(name="ps2", bufs=2, space="PSUM") as ps2,
            tc.tile_pool(name="st2", bufs=3) as st2,
            tc.tile_pool(name="outp", bufs=3) as outp,
        ):
            wl1 = cpool.tile([D, 2 * D], mm_dt)
            wg = cpool.tile([D, 8 * D], mm_dt)
            bias = cpool.tile([D, 8], f32)
            nc.sync.dma_start(wl1[:], wl1_d[:])
            nc.sync.dma_start(wg[:], wg_d[:])
            nc.sync.dma_start(bias[:], bias_d[:])

            for _rep in range(reps):
              h1r = slab.tile([D, LH], mm_dt, tag="h1r")
              h1i = slab.tile([D, LH], mm_dt, tag="h1i")

              n1 = (LH + L1_CHUNK - 1) // L1_CHUNK
              for k in range(n1):
                  s = k * L1_CHUNK
                  cw = min(L1_CHUNK, LH - s)
                  xt = xs.tile([D, L1_CHUNK], mm_dt)
                  nc.sync.dma_start(xt[:, :cw], xT[:, s : s + cw])
                  pr = ps1.tile([D, L1_CHUNK], f32, tag="ps1r")
                  pi = ps1.tile([D, L1_CHUNK], f32, tag="ps1i")
                  nc.tensor.matmul(
                      pr[:, :cw], wl1[:, 0:D], xt[:, :cw], start=True, stop=True
                  )
                  nc.tensor.matmul(
                      pi[:, :cw], wl1[:, D : 2 * D], xt[:, :cw],
                      start=True, stop=True,
                  )
                  nc.scalar.activation(
                      h1r[:, s : s + cw], pr[:, :cw], AF.Relu, bias=bias[:, 0:1]
                  )
                  nc.scalar.activation(
                      h1i[:, s : s + cw], pi[:, :cw], AF.Relu, bias=bias[:, 1:2]
                  )

              n2 = (L + CHUNK - 1) // CHUNK
              for k in range(n2):
                  a = HL + k * CHUNK
                  cw = min(CHUNK, L - k * CHUNK)
                  w2n = cw + 17 + ((cw + 17) % 2)
                  w1s, w1n = a - 18, w2n + 17
                  w2s = a - 9

                  dve_view = (
                      (lambda ap: ap.bitcast(f32))
                      if matmul_dtype == "float32r"
                      else (lambda ap: ap)
                  )
                  p1r = st2.tile([D, CHUNK + 36], f32, tag="p1r")
                  p1i = st2.tile([D, CHUNK + 36], f32, tag="p1i")
                  nc.vector.tensor_tensor_scan(
                      p1r[:, :w1n],
                      dve_view(h1r[:, w1s : w1s + w1n]),
                      dve_view(h1r[:, w1s : w1s + w1n]),
                      0.0, OP.add, OP.bypass,
                  )
                  nc.vector.tensor_tensor_scan(
                      p1i[:, :w1n],
                      dve_view(h1i[:, w1s : w1s + w1n]),
                      dve_view(h1i[:, w1s : w1s + w1n]),
                      0.0, OP.add, OP.bypass,
                  )
                  ns1r = st2.tile([D, CHUNK + 18], mm_dt, tag="ns1r")
                  ns1i = st2.tile([D, CHUNK + 18], mm_dt, tag="ns1i")
                  nc.vector.scalar_tensor_tensor(
                      ns1r[:, :w2n], p1r[:, 17 : 17 + w2n], 1.0, p1r[:, 0:w2n],
                      OP.mult, OP.subtract,
                  )
                  nc.vector.scalar_tensor_tensor(
                      ns1i[:, :w2n], p1i[:, 17 : 17 + w2n], 1.0, p1i[:, 0:w2n],
                      OP.mult, OP.subtract,
                  )

                  pgr = ps2.tile([D, CHUNK + 18], f32, tag="ps2r")
                  pgi = ps2.tile([D, CHUNK + 18], f32, tag="ps2i")
                  rhs_list = [
                      h1r[:, w2s : w2s + w2n],
                      h1i[:, w2s : w2s + w2n],
                      ns1r[:, :w2n],
                      ns1i[:, :w2n],
                  ]
                  for comp, ptile in ((0, pgr), (1, pgi)):
                      for t_i, rhs in enumerate(rhs_list):
                          wcol = (comp * 4 + t_i) * D
                          nc.tensor.matmul(
                              ptile[:, :w2n], wg[:, wcol : wcol + D], rhs,
                              start=(t_i == 0), stop=(t_i == 3),
                          )

                  h2r = st2.tile([D, CHUNK + 18], f32, tag="h2r")
                  h2i = st2.tile([D, CHUNK + 18], f32, tag="h2i")
                  nc.scalar.activation(
                      h2r[:, :w2n], pgr[:, :w2n], AF.Relu, bias=bias[:, 2:3]
                  )
                  nc.scalar.activation(
                      h2i[:, :w2n], pgi[:, :w2n], AF.Relu, bias=bias[:, 3:4]
                  )

                  sc = _build_program.scalars
                  h2c = st2.tile([D, CHUNK + 18], f32, tag="h2c")
                  if sc["h2c_on_r"]:
                      nc.vector.scalar_tensor_tensor(
                          h2c[:, :w2n], h2r[:, :w2n], sc["h2c_scale"],
                          h2i[:, :w2n], OP.mult, OP.add,
                      )
                  else:
                      nc.vector.scalar_tensor_tensor(
                          h2c[:, :w2n], h2i[:, :w2n], sc["h2c_scale"],
                          h2r[:, :w2n], OP.mult, OP.add,
                      )
                  u = st2.tile([D, CHUNK + 18], f32, tag="u")
                  nc.vector.tensor_tensor_scan(
                      u[:, :w2n], h2c[:, :w2n], h2c[:, :w2n], 0.0,
                      OP.add, OP.bypass,
                  )
                  t1 = st2.tile([D, CHUNK], f32, tag="t1")
                  t2 = st2.tile([D, CHUNK], f32, tag="t2")
                  t3 = st2.tile([D, CHUNK], f32, tag="t3")
                  ot = outp.tile([D, CHUNK], f32)
                  nc.vector.scalar_tensor_tensor(
                      t1[:, :cw], h2i[:, 9 : 9 + cw], sc["r1"],
                      h2r[:, 9 : 9 + cw], OP.mult, OP.add,
                  )
                  nc.vector.scalar_tensor_tensor(
                      t2[:, :cw], u[:, 17 : 17 + cw], sc["qn"], t1[:, :cw],
                      OP.mult, OP.add,
                  )
                  nc.vector.scalar_tensor_tensor(
                      t3[:, :cw], u[:, 0:cw], -sc["qn"], t2[:, :cw],
                      OP.mult, OP.add,
                  )
                  nc.vector.tensor_scalar(
                      ot[:, :cw], t3[:, :cw], sc["c0"], bias[:, 4:5],
                      OP.mult, OP.add,
                  )
                  nc.sync.dma_start(outT[:, k * CHUNK : k * CHUNK + cw], ot[:, :cw])

    nc.compile()
    return nc


_MM_DTYPE = "float32r"


# ------------------------------------------------------- cached PJRT runner
_EXECS = {}   # id(nc) -> (sharded_fn, in_names, out_names, out_avals)
_DEVCACHE = {}  # name -> (bytes_key, device_array)
_ZEROS = {}   # (shape, dtype-str) -> device array


def _get_executable(nc):
    global _EXECS
    key = id(nc)
    if key in _EXECS:
        return _EXECS[key]
    import jax
    import numpy as _np
    from jax.sharding import Mesh, PartitionSpec
    from jax.experimental.shard_map import shard_map

    import concourse.mybir as mybir
    from concourse import bass2jax

    bass2jax.install_neuronx_cc_hook()

    partition_name = (
        nc.partition_id_tensor.name if nc.partition_id_tensor else None
    )
    in_names, out_names, out_avals = [], [], []
    for alloc in nc.m.functions[0].allocations:
        if not isinstance(alloc, mybir.MemoryLocationSet):
            continue
        name = alloc.memorylocations[0].name
        if alloc.kind == "ExternalInput":
            if name != partition_name:
                in_names.append(name)
        elif alloc.kind == "ExternalOutput":
            out_names.append(name)
            out_avals.append(
                jax.core.ShapedArray(
                    tuple(alloc.tensor_shape), mybir.dt.np(alloc.dtype)
                )
            )
    all_names = in_names + out_names
    if partition_name is not None:
        all_names = all_names + [partition_name]

    def _body(*args):
        operands = list(args)
        if partition_name is not None:
            operands.append(bass2jax.partition_id_tensor())
        outs = bass2jax._bass_exec_p.bind(
            *operands,
            out_avals=tuple(out_avals),
            in_names=tuple(all_names),
            out_names=tuple(out_names),
            lowering_input_output_aliases=(),
            sim_require_finite=True,
            sim_require_nnan=True,
            nc=nc,
        )
        return tuple(outs)

    devices = jax.devices()[:NCORES]
    mesh = Mesh(_np.asarray(devices), ("core",))
    n_in = len(in_names) + len(out_names)
    sharded = jax.jit(
        shard_map(
            _body,
            mesh=mesh,
            in_specs=(PartitionSpec("core"),) * n_in,
            out_specs=(PartitionSpec("core"),) * len(out_names),
            check_rep=False,
        ),
        keep_unused=True,
    )
    _EXECS[key] = (sharded, in_names, out_names, out_avals)
    return _EXECS[key]


def _sharding():
    import jax
    from jax.sharding import Mesh, NamedSharding, PartitionSpec

    mesh = Mesh(np.asarray(jax.devices()[:NCORES]), ("core",))
    return NamedSharding(mesh, PartitionSpec("core"))


def _dev_cached(name, arr):
    """Device-put arr (replicated-per-core concat) unless bytes unchanged."""
    import jax

    key = arr.tobytes()
    hit = _DEVCACHE.get(name)
    if hit is not None and hit[0] == key:
        return hit[1]
    dev = jax.device_put(arr, _sharding())
    jax.block_until_ready(dev)
    _DEVCACHE[name] = (key, dev)
    return dev


def _zeros_for(aval):
    import jax
    import jax.numpy as jnp

    key = (tuple(aval.shape), str(aval.dtype))
    if key not in _ZEROS:
        z = jax.device_put(
            jnp.zeros((NCORES * aval.shape[0], *aval.shape[1:]), aval.dtype),
            _sharding(),
        )
        jax.block_until_ready(z)
        _ZEROS[key] = z
    return _ZEROS[key]


def _execute(nc, arrays):
    """Run nc with named concat input arrays; returns dict of host outputs."""
    sharded, in_names, out_names, out_avals = _get_executable(nc)
    args = []
    for name in in_names:
        a = arrays[name]
        if name in ("wz", "bz", "wl1", "wg", "biases"):
            args.append(_dev_cached(name, a))
        else:
            args.append(a)
    zeros = [_zeros_for(a) for a in out_avals]
    out_arrs = sharded(*args, *zeros)
    return {
        name: np.asarray(out_arrs[i]).reshape(NCORES, *out_avals[i].shape)
        for i, name in enumerate(out_names)
    }


# ---------------------------------------------------------------- v2 runner
def _get_program2(scalars):
    global _P2, _P2_scalars
    if _P2 is None or _P2_scalars != scalars:
        _P2 = _build_program2(scalars)
        _P2_scalars = dict(scalars)
    return _P2


def _prepare2(ins):
    import ml_dtypes

    bf16 = np.dtype(ml_dtypes.bfloat16)
    w1 = _evolution_row(DEG, float(ins["t1r"]), float(ins["t1i"]))
    w2 = _evolution_row(DEG, float(ins["t2r"]), float(ins["t2i"]))
    wz, bz, scalars = _fold2(ins, w1, w2)

    x = ins["x"]
    xbf = x.astype(bf16)
    xs_all = np.empty((NCORES * LH2, D), bf16)
    for c in range(NCORES):
        v = xs_all[c * LH2 : (c + 1) * LH2]
        lo = c * L
        v[HL2 : HL2 + L] = xbf[lo : lo + L]
        if lo >= HL2:
            v[:HL2] = xbf[lo - HL2 : lo]
        else:
            v[:HL2] = xbf[N - HL2 :]
        hi = lo + L
        if hi + HR2 <= N:
            v[HL2 + L :] = xbf[hi : hi + HR2]
        else:
            v[HL2 + L :] = xbf[: HR2]
    arrays = {
        "xs": xs_all,
        "wz": np.broadcast_to(wz, (NCORES, *wz.shape)).reshape(NCORES * D, -1),
        "bz": np.broadcast_to(bz, (NCORES, *bz.shape)).reshape(NCORES * D, -1),
    }
    arrays = {k: np.ascontiguousarray(a) for k, a in arrays.items()}
    return arrays, scalars


def _run_v2(ins):
    arrays, scalars = _prepare2(ins)
    nc = _get_program2(scalars)
    outs = _execute(nc, arrays)
    return np.ascontiguousarray(
        outs["outN"].reshape(N, D).astype(np.float32)
    )


# ------------------------------------------------------------- legacy runner
def _get_program(scalars):
    global _PROGRAM
    _build_program.scalars = scalars
    if _PROGRAM is None:
        _PROGRAM = _build_program(_MM_DTYPE)
    return _PROGRAM


def _reset_program(mm_dtype):
    global _MM_DTYPE, _PROGRAM
    _MM_DTYPE = mm_dtype
    _PROGRAM = None


def _prepare(ins):
    w1 = _evolution_row(DEG, float(ins["t1r"]), float(ins["t1i"]))
    w2 = _evolution_row(DEG, float(ins["t2r"]), float(ins["t2i"]))
    wl1, wg, biases, scalars = _fold_weights(ins, w1, w2)
    _get_program(scalars)

    x = ins["x"].astype(np.float32, copy=False)
    idx = np.arange(-HL, L + HR)
    xs_parts = []
    for c in range(NCORES):
        rows = (c * L + idx) % N
        xs_parts.append(np.ascontiguousarray(x[rows].T))
    arrays = {
        "xT": np.concatenate(xs_parts, axis=0),
        "wl1": np.broadcast_to(wl1, (NCORES, *wl1.shape)).reshape(NCORES * D, -1),
        "wg": np.broadcast_to(wg, (NCORES, *wg.shape)).reshape(NCORES * D, -1),
        "biases": np.broadcast_to(biases, (NCORES, *biases.shape)).reshape(
            NCORES * D, -1
        ),
    }
    arrays = {k: np.ascontiguousarray(a) for k, a in arrays.items()}
    return arrays


def _run_legacy(ins):
    arrays = _prepare(ins)
    outs = _execute(_PROGRAM, arrays)
    out = np.empty((N, D), np.float32)
    for c in range(NCORES):
        out[c * L : (c + 1) * L] = outs["outT"][c].T
    return out


# ---------------------------------------------------------------- entrypoint
def _run(ins, trace=False):
    ins = {k: np.asarray(v) for k, v in ins.items()}
    if not _is_circulant(ins["edge_index"]):
        return _fallback_numpy(ins), None
    try:
        return _run_v2(ins), None
    except _Degenerate:
        return _fallback_numpy(ins), None
    except Exception:
        import traceback

        traceback.print_exc()
    try:
        return _run_legacy(ins), None
    except Exception:
        if _MM_DTYPE == "float32":
            raise
        _reset_program("float32")
        return _run_legacy(ins), None


def kernel(**inputs):
    out, _ = _run(inputs)
    return out
